# revision 41
# baseline (speedup 1.0000x reference)
"""GroupedQueryAttention on 8 Trainium2 NeuronCores.

Sharding: core c = 4*b + g handles batch b (of 2) and KV group g (of 4),
i.e. 4 query heads (512 q-dims) + one 128-dim K/V head. o_proj is computed
as per-group partials and summed with per-half-t-chunk fp16 ReduceScatters
across the 4 cores of each batch, pipelined against compute; each core ends
up with a d-band of out^T over all T, which the host reassembles.

All matmuls run in fp16 (fp8 was tried and rejected: its quantization
noise exceeds the 2e-2 max-error budget) with fp32 PSUM accumulation.
Layouts avoid any transpose of the big P matrix:
  - projections produce Q^T/K^T directly (lhsT=W tile, rhs=x^T tile)
  - scores are computed as S^T = (K^T).T @ Q^T
  - exp(S^T) = P^T feeds A@V as lhsT directly
  - V carries an extra ones-column so the softmax denominator falls out
    of the A@V matmul for free; normalization applies to the small A@V
    output rather than to P.
The attention inner loop is software-pipelined (scores 3 s-blocks ahead
of A@V) with previous-chunk o_proj matmuls interleaved one-per-step as
exp-independent PE filler; each ReduceScatter half fires as soon as its
8 partial blocks are written, and only an idle-queue DRAM copy consumes
RS output, so no hot engine queue ever blocks on a collective.
"""

import math
import sys

import numpy as np

sys.path.insert(0, "/opt/trn_rl_repo")

B = 2
T = 2048
D = 2048
HEADS = 16
GROUPS = 4
HD = 128  # head dim
M = HEADS // GROUPS  # heads per group = 4
GQ = M * HD  # q dims per group = 512
SCALE = 1.0 / math.sqrt(HD)
N_CORES = 8
TCH = 512  # t chunk
NTCH = T // TCH  # 4
NSB = T // 128  # 16 s blocks
NKS = D // 128  # 16 contraction steps for projections

_COMPILED = {}


def _build():
    import concourse.bass as bass
    import concourse.mybir as mybir
    import concourse.tile as tile
    from concourse import bacc
    from concourse.masks import make_identity

    f16 = mybir.dt.float16
    f32 = mybir.dt.float32
    Exp = mybir.ActivationFunctionType.Exp

    nc = bacc.Bacc("TRN2", target_bir_lowering=False, num_devices=N_CORES)

    xT = nc.declare_dram_parameter("xT", [D, T], f16, isOutput=False)
    wq = nc.declare_dram_parameter("wq", [D, GQ], f16, isOutput=False)
    wk = nc.declare_dram_parameter("wk", [D, HD], f16, isOutput=False)
    wv = nc.declare_dram_parameter("wv", [D, HD], f16, isOutput=False)
    wo = nc.declare_dram_parameter("wo", [GQ, D], f16, isOutput=False)
    bqs_d = nc.declare_dram_parameter("bqs", [128, M], f32, isOutput=False)
    bks_d = nc.declare_dram_parameter("bks", [128, 1], f32, isOutput=False)
    bvs_d = nc.declare_dram_parameter("bvs", [128, 1], f32, isOutput=False)
    bo4_d = nc.declare_dram_parameter("bo4", [128, D // 128], f32, isOutput=False)
    # core (b, j) outputs, per (t-chunk, half): out^T rows
    # [half*1024 + j*256, +256) for t in the chunk (fp16, from the RS)
    outT = nc.declare_dram_parameter(
        "outT", [NTCH, 2, TCH // 2, TCH], f16, isOutput=True
    )

    groups = [[0, 1, 2, 3], [4, 5, 6, 7]]

    with tile.TileContext(nc) as tc:
        with (
            tc.tile_pool(name="const", bufs=1) as const,
            tc.tile_pool(name="work", bufs=2) as work,
            tc.tile_pool(name="psum", bufs=1, space="PSUM") as psum,
            tc.tile_pool(name="dram", bufs=1, space="DRAM") as dram,
        ):
            ident = const.tile([128, 128], f16)
            make_identity(nc, ident)
            bqs = const.tile([128, M], f32)
            bks = const.tile([128, 1], f32)
            bvs = const.tile([128, 1], f32)
            bo4 = const.tile([128, D // 128], f32)
            nc.scalar.dma_start(bqs[:], bqs_d[:])
            nc.scalar.dma_start(bks[:], bks_d[:])
            nc.scalar.dma_start(bvs[:], bvs_d[:])
            nc.scalar.dma_start(bo4[:], bo4_d[:])

            xt = const.tile([128, NKS, T], f16)
            wq_sb = const.tile([128, NKS, GQ], f16)
            wk_sb = const.tile([128, NKS, HD], f16)
            wv_sb = const.tile([128, NKS, HD], f16)
            wo_sb = const.tile([128, M, D], f16)
            qs = [nc.sync, nc.scalar]
            for i in range(NKS):
                qs[i % 2].dma_start(wk_sb[:, i, :], wk[i * 128 : (i + 1) * 128, :])
                qs[(i + 1) % 2].dma_start(
                    xt[:, i, :], xT[i * 128 : (i + 1) * 128, :]
                )
            for i in range(NKS):
                qs[i % 2].dma_start(wv_sb[:, i, :], wv[i * 128 : (i + 1) * 128, :])
                qs[(i + 1) % 2].dma_start(
                    wq_sb[:, i, :], wq[i * 128 : (i + 1) * 128, :]
                )
            for h in range(M):
                qs[h % 2].dma_start(wo_sb[:, h, :], wo[h * 128 : (h + 1) * 128, :])

            qt = const.tile([128, M, T], f16)
            kt = const.tile([128, T], f16)
            vt_sb = const.tile([128, T], f16)
            v_sb = const.tile([128, NSB, 132], f16)

            # ---- projections ----
            # ks-outer over groups of up to 3 t-chunks: each weight block
            # stays resident in the PE across the group's matmuls
            def proj_group(w_sb, cols, tcs, emit_out):
                accs = {}
                for tc_i in tcs:
                    accs[tc_i] = psum.tile(
                        [128, TCH], f32, tag="acc", bufs=3, name="acc"
                    )
                for ks in range(NKS):
                    for tc_i in tcs:
                        nc.tensor.matmul(
                            accs[tc_i][:],
                            w_sb[:, ks, cols],
                            xt[:, ks, tc_i * TCH : (tc_i + 1) * TCH],
                            start=(ks == 0),
                            stop=(ks == NKS - 1),
                        )
                for tc_i in tcs:
                    emit_out(tc_i, accs[tc_i])

            def k_out(tc_i, acc):
                nc.vector.tensor_scalar_add(
                    kt[:, tc_i * TCH : (tc_i + 1) * TCH], acc[:], bks[:, 0:1]
                )

            def v_out(tc_i, acc):
                nc.vector.tensor_scalar_add(
                    vt_sb[:, tc_i * TCH : (tc_i + 1) * TCH], acc[:], bvs[:, 0:1]
                )

            proj_group(wk_sb, slice(0, HD), [0, 1, 2], k_out)
            proj_group(wk_sb, slice(0, HD), [3], k_out)
            proj_group(wv_sb, slice(0, HD), [0, 1, 2], v_out)
            proj_group(wv_sb, slice(0, HD), [3], v_out)
            # V natural [s, hd] + ones column; transposes borrow banks
            # from the (idle) opk tag ring
            for s in range(NSB):
                tp = psum.tile([128, 128], f16, tag="opk", bufs=4, name="tp")
                nc.tensor.transpose(tp[:], vt_sb[:, s * 128 : (s + 1) * 128], ident[:])
                nc.vector.tensor_copy(v_sb[:, s, 0:128], tp[:])
            nc.vector.memset(v_sb[:, :, 128:129], 1.0)
            q_out_fns = []
            for h in range(M):

                def q_out(tc_i, acc, h=h):
                    nc.vector.tensor_scalar(
                        qt[:, h, tc_i * TCH : (tc_i + 1) * TCH],
                        acc[:],
                        SCALE,
                        bqs[:, h : h + 1],
                        op0=mybir.AluOpType.mult,
                        op1=mybir.AluOpType.add,
                    )

                proj_group(wq_sb, slice(h * 128, (h + 1) * 128), [0, 1, 2], q_out)
                q_out_fns.append(q_out)

            # ---- attention + interleaved o_proj, streamed per t-chunk ----
            partials = [
                dram.tile([D, TCH], f16, tag=f"ptl{i}", name=f"ptl{i}")
                for i in range(NTCH)
            ]
            rss = [
                dram.tile([2, TCH // 2, TCH], f16, tag=f"rs{i}", name=f"rs{i}")
                for i in range(NTCH)
            ]

            def emit_rs_half(tc_j, half):
                # ReduceScatter of one d-half of chunk tc_j (fp16). Only
                # consumer of the RS output is a DRAM->DRAM DMA on the idle
                # gpsimd queue, emitted one chunk late, so hot engine queues
                # never block on the collective.
                nc.gpsimd.collective_compute(
                    "ReduceScatter",
                    mybir.AluOpType.add,
                    replica_groups=groups,
                    ins=[partials[tc_j][half * 1024 : (half + 1) * 1024, :]],
                    outs=[rss[tc_j][half]],
                )

            def make_qproj_jobs():
                """Q projection for the last t-chunk as per-matmul filler
                jobs for the first attention chunk (which has no o_proj
                work yet). Accumulates in the pp bank, idle during tc0."""
                jobs = []
                for h in range(M):
                    holder = {}

                    def qmm(ks, h=h, holder=holder):
                        if ks == 0:
                            holder["pp"] = psum.tile(
                                [128, TCH], f32, tag="pp", bufs=1, name="pp"
                            )
                        acc = holder["pp"]
                        nc.tensor.matmul(
                            acc[:],
                            wq_sb[:, ks, h * 128 : (h + 1) * 128],
                            xt[:, ks, 3 * TCH : 4 * TCH],
                            start=(ks == 0),
                            stop=(ks == NKS - 1),
                        )
                        if ks == NKS - 1:
                            q_out_fns[h](3, acc)

                    for ks in range(NKS):
                        jobs.append(lambda qmm=qmm, ks=ks: qmm(ks))
                    jobs.append(None)  # drain gap for the pp bank
                return jobs

            def make_oproj_jobs(tc_j, at_tile):
                """o_proj for chunk tc_j as per-PE-matmul closures,
                interleaved into the next chunk's attention as exp-independent
                PE filler. The 4th matmul of each cb also emits bias-add +
                partial DMA; a None gap after each cb lets the single pp bank
                drain. The RS for each d-half fires as soon as its 8 cb
                partials are written."""
                jobs = []
                partial = partials[tc_j]
                for cb in range(D // 128):
                    holder = {}

                    def mm(hh, cb=cb, holder=holder, at_tile=at_tile,
                           partial=partial, tc_j=tc_j):
                        if hh == 0:
                            holder["pp"] = psum.tile(
                                [128, TCH], f32, tag="pp", bufs=1, name="pp"
                            )
                        ppt = holder["pp"]
                        nc.tensor.matmul(
                            ppt[:],
                            wo_sb[:, hh, cb * 128 : (cb + 1) * 128],
                            at_tile[:, hh, :],
                            start=(hh == 0),
                            stop=(hh == M - 1),
                        )
                        if hh == M - 1:
                            po_sb = work.tile(
                                [128, TCH], f16, tag="po", bufs=4, name="po_sb"
                            )
                            nc.vector.tensor_scalar_add(
                                po_sb[:], ppt[:], bo4[:, cb : cb + 1]
                            )
                            nc.sync.dma_start(
                                partial[cb * 128 : (cb + 1) * 128, :], po_sb[:]
                            )
                            if cb == D // 256 - 1:
                                emit_rs_half(tc_j, 0)
                            elif cb == D // 128 - 1:
                                emit_rs_half(tc_j, 1)

                    for hh in range(M):
                        jobs.append(lambda mm=mm, hh=hh: mm(hh))
                    jobs.append(None)  # drain gap for the pp bank
                return jobs

            prev_at = None
            for tc_i in range(NTCH):
                at = work.tile([128, M, TCH], f16, tag="at", bufs=2, name="at")
                ojobs = (
                    make_oproj_jobs(tc_i - 1, prev_at)
                    if tc_i > 0
                    else make_qproj_jobs()
                )
                for h in range(M):
                    # one PSUM bank per accumulator: a matmul start zeroes the
                    # whole 2KB bank, so groups can never share a bank
                    opks = [
                        psum.tile([128, 129], f32, tag="opk", bufs=4, name=f"opk{i}")
                        for i in range(4)
                    ]

                    sps_l = [None] * NSB
                    p_l = [None] * NSB

                    def emit_score(s, h=h, tc_i=tc_i, sps_l=sps_l):
                        sps = psum.tile([128, TCH], f32, tag="acc", bufs=3, name="sps")
                        nc.tensor.matmul(
                            sps[:],
                            kt[:, s * 128 : (s + 1) * 128],
                            qt[:, h, tc_i * TCH : (tc_i + 1) * TCH],
                            start=True,
                            stop=True,
                        )
                        sps_l[s] = sps

                    def emit_exp(s, sps_l=sps_l, p_l=p_l):
                        p_sb = work.tile([128, TCH], f16, tag="p", bufs=6, name="p_sb")
                        nc.scalar.activation(p_sb[:], sps_l[s][:], Exp)
                        p_l[s] = p_sb
                        sps_l[s] = None

                    def emit_av(s, opks=opks, p_l=p_l):
                        p_sb = p_l[s]
                        for tb in range(4):
                            nc.tensor.matmul(
                                opks[tb][:, 0:129],
                                p_sb[:, tb * 128 : (tb + 1) * 128],
                                v_sb[:, s, 0:129],
                                start=(s == 0),
                                stop=(s == NSB - 1),
                            )
                        p_l[s] = None

                    # software pipeline: scores run 3 s-blocks ahead of
                    # A@V; the (always-ready) previous-chunk o_proj filler
                    # goes first in each step so a stalled score never
                    # blocks it in the in-order PE queue
                    DEPTH = 3
                    for s in range(NSB + DEPTH):
                        if ojobs:
                            job = ojobs.pop(0)
                            if job is not None:
                                job()
                        if s < NSB:
                            emit_score(s)
                            emit_exp(s)
                        elif ojobs:
                            # epilogue step: no score to issue, room for a
                            # second filler job
                            job = ojobs.pop(0)
                            if job is not None:
                                job()
                        if s >= DEPTH:
                            emit_av(s - DEPTH)

                    # normalize head output and transpose into at; the
                    # transposes borrow the just-freed opk banks
                    for tb in range(4):
                        opk = opks[tb]
                        rcp = work.tile([128, 1], f32, tag="rcp", bufs=4, name="rcp")
                        nc.vector.reciprocal(rcp[:], opk[:, 128:129])
                        o_sb = work.tile([128, 128], f16, tag="osb", bufs=4, name="osb")
                        nc.vector.tensor_scalar_mul(o_sb[:], opk[:, 0:128], rcp[:])
                        tp = psum.tile([128, 128], f16, tag="opk", bufs=4, name="tp")
                        nc.tensor.transpose(tp[:], o_sb[:], ident[:])
                        nc.vector.tensor_copy(
                            at[:, h, tb * 128 : (tb + 1) * 128], tp[:]
                        )
                        if ojobs:
                            job = ojobs.pop(0)
                            if job is not None:
                                job()
                while ojobs:
                    job = ojobs.pop(0)
                    if job is not None:
                        job()
                if tc_i > 0:
                    nc.gpsimd.dma_start(outT[tc_i - 1], rss[tc_i - 1][:])
                prev_at = at

            # tail: o_proj + RS halves for the last chunk
            for job in make_oproj_jobs(NTCH - 1, prev_at):
                if job is not None:
                    job()
            nc.gpsimd.dma_start(outT[NTCH - 1], rss[NTCH - 1][:])

    nc.compile()
    return nc


def _get_nc():
    if "nc" not in _COMPILED:
        _COMPILED["nc"] = _build()
    return _COMPILED["nc"]


def _make_in_maps(x, Wq, bq, Wk, bk, Wv, bv, Wo, bo):
    import ml_dtypes

    f8np = ml_dtypes.float8_e4m3

    x = np.asarray(x, np.float32)
    Wq = np.asarray(Wq, np.float32)
    Wk = np.asarray(Wk, np.float32)
    Wv = np.asarray(Wv, np.float32)
    Wo = np.asarray(Wo, np.float32)
    bq = np.asarray(bq, np.float32)
    bk = np.asarray(bk, np.float32)
    bv = np.asarray(bv, np.float32)
    bo = np.asarray(bo, np.float32)

    in_maps = []
    for c in range(N_CORES):
        b, g = c // 4, c % 4
        in_maps.append(
            {
                "xT": np.ascontiguousarray(x[b].T).astype(np.float16),
                "wq": np.ascontiguousarray(
                    Wq[:, g * GQ : (g + 1) * GQ]
                ).astype(np.float16),
                "wk": np.ascontiguousarray(
                    Wk[:, g * HD : (g + 1) * HD]
                ).astype(np.float16),
                "wv": np.ascontiguousarray(
                    Wv[:, g * HD : (g + 1) * HD]
                ).astype(np.float16),
                "wo": np.ascontiguousarray(
                    Wo[g * GQ : (g + 1) * GQ, :]
                ).astype(np.float16),
                "bqs": np.ascontiguousarray(
                    (bq[g * GQ : (g + 1) * GQ] * SCALE).reshape(M, 128).T
                ),
                "bks": np.ascontiguousarray(
                    bk[g * HD : (g + 1) * HD].reshape(1, 128).T
                ),
                "bvs": np.ascontiguousarray(
                    bv[g * HD : (g + 1) * HD].reshape(1, 128).T
                ),
                "bo4": np.ascontiguousarray((bo / 4.0).reshape(D // 128, 128).T),
            }
        )
    return in_maps


def _assemble(outTs):
    """outTs: list of per-core outT arrays [4, 2, 256, 512] fp16."""
    out = np.empty((B, T, D), np.float32)
    for b in range(B):
        for j in range(4):
            r = outTs[4 * b + j]
            for tc_i in range(NTCH):
                for half in range(2):
                    d0 = half * 1024 + j * 256
                    out[b, tc_i * TCH : (tc_i + 1) * TCH, d0 : d0 + 256] = r[
                        tc_i, half
                    ].T
    return out


def kernel(x, Wq, bq, Wk, bk, Wv, bv, Wo, bo):
    from concourse.bass_utils import run_bass_kernel_spmd

    nc = _get_nc()
    in_maps = _make_in_maps(x, Wq, bq, Wk, bk, Wv, bv, Wo, bo)
    res = run_bass_kernel_spmd(nc, in_maps, list(range(N_CORES)))
    _COMPILED["last_res"] = res
    return _assemble([res.results[c]["outT"] for c in range(N_CORES)])
